# revision 1
# baseline (speedup 1.0000x reference)
"""Trainium2 Bass kernel for the O2O classification head (GNN message passing).

Strategy
--------
The reference edge tensor is rank-structured:
    edge[b,i,j,:] = (f_in_i + pos_i@W_pos + b_in + b_pos) - (f_out_j + b_out + pos_j@W_pos)
                  = A_i - C_j
so after the first edge MLP layer the pre-gelu values are p_i - q_j + b_e1 with
p = A@W_e1, q = C@W_e1 computed once per node.  The [B,N,N,128] edge tensor is
never materialized; each (i,j) pair costs one 128-wide gelu + dot with W_e2.

Host-side, nodes are sorted by (cls desc, id desc).  Then
    suppress[i,j] != 0  requires  rank_i < rank_j
so for a j-tile only the i-prefix [0, rank_max) contributes; everything else is
masked to zero exactly as in the reference (the max always sees explicit zeros,
e.g. at i == j).

Sharding: 2 cores per batch.  Each core takes the 32-wide j-blocks of one
parity (global block 2t+P for t = 0..7) with i-prefix length 64*(t+1) — every
core runs an identical program; all per-core variation is input data.
"""

import sys
import numpy as np

if "/opt/trn_rl_repo" not in sys.path:
    sys.path.insert(0, "/opt/trn_rl_repo")

B, N = 4, 512
H_DIM, I_DIM = 64, 128
N_CORES = 8
N_TILES = 8          # j-tiles per core, 32 j's each
TJ = 32              # j's per tile
ILEN = [64 * (t + 1) for t in range(N_TILES)]   # i-prefix per tile
F32 = np.float32

IMG_W, IMG_H, CENTER_H = 800.0, 320.0, 160.0
NUM_OFFSETS = 72
CONF_THRES = 0.4

_PROGRAM = None  # cached compiled program

INPUT_SPECS = [
    ("bfT_i", (H_DIM, N)),
    ("posT_i", (2, N)),
    ("bfT_j", (H_DIM, 256)),
    ("posT_j", (2, 256)),
    ("angrow", (1, N)),
    ("angcol", (128, 2)),
    ("rankcol", (128, 2)),
    ("iota", (1, N)),
    ("we2d", (128, 32 * 32)),
    ("cls_loc", (1, 256)),
    ("W_cls", (64, 64)),
    ("bcls", (64, 1)),
    ("W_in", (64, 128)),
    ("W_out", (64, 128)),
    ("W_pos", (2, 128)),
    ("bpos", (128, 1)),
    ("W_e1", (128, 128)),
    ("be1", (128, 1)),
    ("we2", (128, 1)),
    ("be2c", (128, 1)),
    ("W_n1", (1, 64)),
    ("bn1", (64, 1)),
    ("W_n2", (64, 64)),
    ("bn2", (64, 1)),
    ("W_head", (64, 1)),
    ("bh", (1, 1)),
]


def _build_program(stage=99, num_devices=N_CORES):
    import contextlib
    import concourse.bass as bass  # noqa: F401
    import concourse.tile as tile
    from concourse import bacc, mybir

    f32 = mybir.dt.float32
    AF = mybir.ActivationFunctionType
    OP = mybir.AluOpType
    AX = mybir.AxisListType

    nc = bacc.Bacc("TRN2", target_bir_lowering=False, debug=False,
                   num_devices=num_devices)

    dram = {}
    for nm, shape in INPUT_SPECS:
        dram[nm] = nc.declare_dram_parameter(nm, list(shape), f32, isOutput=False)
    y = nc.declare_dram_parameter("y", [1, 256], f32, isOutput=True)

    with tile.TileContext(nc) as tc:
        with contextlib.ExitStack() as ctx:
            const = ctx.enter_context(tc.tile_pool(name="const", bufs=1))
            work = ctx.enter_context(tc.tile_pool(name="work", bufs=2))
            upool = ctx.enter_context(tc.tile_pool(name="upool", bufs=2))
            gpool = ctx.enter_context(tc.tile_pool(name="gpool", bufs=2))
            pps = ctx.enter_context(tc.tile_pool(name="pps", bufs=2, space="PSUM"))
            spsum = ctx.enter_context(tc.tile_pool(name="spsum", bufs=3,
                                                   space="PSUM"))

            sb = {}
            for nm, shape in INPUT_SPECS:
                t = const.tile(list(shape), f32, name=f"sb_{nm}", tag=f"sb_{nm}")
                nc.gpsimd.dma_start(out=t[:], in_=dram[nm][:])
                sb[nm] = t

            ones128 = const.tile([1, 128], f32, name="ones128", tag="ones128")
            nc.vector.memset(ones128[:], 1.0)

            def emit_dbg(src_ap):
                dbg = work.tile([1, 256], f32, name="dbg", tag="dbg")
                nc.vector.tensor_copy(dbg[:], src_ap)
                nc.gpsimd.dma_start(out=y[:], in_=dbg[:])

            if stage < 1:
                emit_dbg(sb["cls_loc"][:])

            if stage >= 1:
                # ---------- i-side preprocessing (global sorted order) ------
                ps_f = pps.tile([64, N], f32, name="ps_f", tag="ps")
                nc.tensor.matmul(ps_f[:], sb["W_cls"][:], sb["bfT_i"][:],
                                 start=True, stop=True)
                featsT_i = const.tile([64, N], f32, name="featsT_i",
                                      tag="featsT_i")
                nc.vector.tensor_scalar(featsT_i[:], ps_f[:], sb["bcls"][:],
                                        0.0, OP.add, OP.max)

                ps_A = pps.tile([128, N], f32, name="ps_A", tag="ps")
                nc.tensor.matmul(ps_A[:], sb["W_in"][:], featsT_i[:],
                                 start=True, stop=False)
                nc.tensor.matmul(ps_A[:], sb["W_pos"][:], sb["posT_i"][:],
                                 start=False, stop=True)
                A_T = const.tile([128, N], f32, name="A_T", tag="A_T")
                nc.vector.tensor_scalar_add(A_T[:], ps_A[:], sb["bpos"][:])

                ps_p = pps.tile([128, N], f32, name="ps_p", tag="ps")
                nc.tensor.matmul(ps_p[:], sb["W_e1"][:], A_T[:],
                                 start=True, stop=True)
                p_T = const.tile([128, N], f32, name="p_T", tag="p_T")
                nc.vector.tensor_copy(p_T[:], ps_p[:])

                # ---------- j-side preprocessing (core-local j order) -------
                ps_fj = pps.tile([64, 256], f32, name="ps_fj", tag="ps")
                nc.tensor.matmul(ps_fj[:], sb["W_cls"][:], sb["bfT_j"][:],
                                 start=True, stop=True)
                featsT_j = const.tile([64, 256], f32, name="featsT_j",
                                      tag="featsT_j")
                nc.vector.tensor_scalar(featsT_j[:], ps_fj[:], sb["bcls"][:],
                                        0.0, OP.add, OP.max)

                ps_C = pps.tile([128, 256], f32, name="ps_C", tag="ps")
                nc.tensor.matmul(ps_C[:], sb["W_out"][:], featsT_j[:],
                                 start=True, stop=False)
                nc.tensor.matmul(ps_C[:], sb["W_pos"][:], sb["posT_j"][:],
                                 start=False, stop=True)
                C_T = const.tile([128, 256], f32, name="C_T", tag="C_T")
                nc.vector.tensor_copy(C_T[:], ps_C[:])

                ps_q = pps.tile([128, 256], f32, name="ps_q", tag="ps")
                nc.tensor.matmul(ps_q[:], sb["W_e1"][:], C_T[:],
                                 start=True, stop=True)
                qneg = const.tile([128, 256], f32, name="qneg", tag="qneg")
                nc.vector.tensor_scalar(qneg[:], ps_q[:], -1.0, sb["be1"][:],
                                        OP.mult, OP.add)

                if stage < 2:
                    emit_dbg(p_T[0:1, :256])

            if stage >= 2:
                # ---------- suppression masks -------------------------------
                ps_ab = pps.tile([128, N], f32, name="ps_ab", tag="ps")
                nc.tensor.matmul(ps_ab[:], ones128[:], sb["angrow"][:],
                                 start=True, stop=True)
                angb = const.tile([128, N], f32, name="angb", tag="angb")
                nc.vector.tensor_copy(angb[:], ps_ab[:])

                ps_io = pps.tile([128, N], f32, name="ps_io", tag="ps")
                nc.tensor.matmul(ps_io[:], ones128[:], sb["iota"][:],
                                 start=True, stop=True)
                iotab = const.tile([128, N], f32, name="iotab", tag="iotab")
                nc.vector.tensor_copy(iotab[:], ps_io[:])

                masks = []
                for g in range(2):
                    Lg = 256 if g == 0 else 512
                    acol = sb["angcol"][:, g:g + 1]
                    m1 = work.tile([128, Lg], f32, name=f"m1_{g}", tag="mtmp1")
                    nc.vector.tensor_scalar(m1[:], angb[:, :Lg], acol, 0.5,
                                            OP.subtract, OP.is_lt)
                    m2 = work.tile([128, Lg], f32, name=f"m2_{g}", tag="mtmp2")
                    nc.vector.tensor_scalar(m2[:], angb[:, :Lg], acol, -0.5,
                                            OP.subtract, OP.is_gt)
                    tri = work.tile([128, Lg], f32, name=f"tri_{g}", tag="mtmp3")
                    nc.vector.tensor_scalar(tri[:], iotab[:, :Lg],
                                            sb["rankcol"][:, g:g + 1], None,
                                            OP.is_lt)
                    t3 = work.tile([128, Lg], f32, name=f"t3_{g}", tag="mtmp1")
                    nc.vector.tensor_tensor(t3[:], m1[:], m2[:], OP.logical_and)
                    mg = const.tile([128, Lg], f32, name=f"mask{g}",
                                    tag=f"mask{g}")
                    nc.vector.tensor_tensor(mg[:], t3[:], tri[:], OP.logical_and)
                    masks.append(mg)

                if stage < 3:
                    emit_dbg(masks[1][0:1, :256])

            if stage >= 3:
                # ---------- main loop ---------------------------------------
                nmall = const.tile([TJ, N_TILES], f32, name="nmall", tag="nmall")
                n_tiles_run = 1 if stage == 3 else N_TILES
                if stage == 3:
                    nc.vector.memset(nmall[:], 0.0)
                for t in range(n_tiles_run):
                    L = ILEN[t]
                    g, prow = t // 4, TJ * (t % 4)
                    S = spsum.tile([TJ, L], f32, name=f"S_{t}", tag="sbank")
                    for c in range(2):
                        U = upool.tile([128, 16 * L], f32, name=f"U_{t}_{c}",
                                       tag="u")
                        for jj in range(16):
                            l = TJ * t + 16 * c + jj
                            nc.vector.tensor_scalar_add(
                                U[:, jj * L:(jj + 1) * L], p_T[:, :L],
                                qneg[:, l:l + 1])
                        G = gpool.tile([128, 16 * L], f32, name=f"G_{t}_{c}",
                                       tag="g")
                        nc.scalar.activation(G[:], U[:], AF.Gelu)
                        for jj in range(16):
                            r = 16 * c + jj
                            nc.tensor.matmul(S[:, :],
                                             sb["we2d"][:, TJ * r:TJ * (r + 1)],
                                             G[:, jj * L:(jj + 1) * L],
                                             start=(r == 0), stop=(r == TJ - 1))
                    # masked = (S + b_e2) * mask ; node_max = rowmax(masked)
                    msk = work.tile([TJ, L], f32, name=f"msk_{t}", tag="msk")
                    nc.vector.scalar_tensor_tensor(
                        msk[:], S[:], sb["be2c"][prow:prow + TJ],
                        masks[g][prow:prow + TJ, :L], OP.add, OP.mult)
                    nc.vector.reduce_max(nmall[:, t:t + 1], msk[:], axis=AX.X)

                if stage < 5:
                    fl = work.tile([1, 256], f32, name="fl", tag="dbg")
                    nc.gpsimd.dma_start(out=fl[:], in_=nmall[:])
                    nc.gpsimd.dma_start(out=y[:], in_=fl[:])

            if stage >= 5:
                # ---------- final MLP over node_max -------------------------
                # flatten [32, 8] -> [1, 256]; f = 8*pp + q (host unscrambles)
                nm_flat = work.tile([1, 256], f32, name="nm_flat", tag="nm_flat")
                nc.gpsimd.dma_start(out=nm_flat[:], in_=nmall[:])

                ps_h1 = pps.tile([64, 256], f32, name="ps_h1", tag="ps")
                nc.tensor.matmul(ps_h1[:], sb["W_n1"][:], nm_flat[:],
                                 start=True, stop=True)
                s1 = work.tile([64, 256], f32, name="s1", tag="s1")
                nc.vector.tensor_scalar(s1[:], ps_h1[:], sb["bn1"][:], 0.0,
                                        OP.add, OP.max)

                ps_h2 = pps.tile([64, 256], f32, name="ps_h2", tag="ps")
                nc.tensor.matmul(ps_h2[:], sb["W_n2"][:], s1[:],
                                 start=True, stop=True)
                s2 = work.tile([64, 256], f32, name="s2", tag="s2")
                nc.vector.tensor_scalar(s2[:], ps_h2[:], sb["bn2"][:], 0.0,
                                        OP.add, OP.max)

                ps_L0 = pps.tile([1, 256], f32, name="ps_L0", tag="ps")
                nc.tensor.matmul(ps_L0[:], sb["W_head"][:], s2[:],
                                 start=True, stop=True)
                t1 = work.tile([1, 256], f32, name="t1f", tag="t1f")
                nc.vector.tensor_scalar(t1[:], ps_L0[:], sb["bh"][:], 1.0e6,
                                        OP.add, OP.add)
                mker = work.tile([1, 256], f32, name="mker", tag="mker")
                nc.vector.tensor_scalar(mker[:], sb["cls_loc"][:],
                                        float(F32(CONF_THRES)), None, OP.is_ge)
                t2 = work.tile([1, 256], f32, name="t2f", tag="t2f")
                nc.vector.tensor_tensor(t2[:], t1[:], mker[:], OP.mult)
                t3f = work.tile([1, 256], f32, name="t3f", tag="t3f")
                nc.vector.tensor_scalar_add(t3f[:], t2[:], -1.0e6)
                out_t = work.tile([1, 256], f32, name="out_t", tag="out_t")
                nc.scalar.activation(out_t[:], t3f[:], AF.Sigmoid)
                nc.gpsimd.dma_start(out=y[:], in_=out_t[:])

    nc.compile()
    return nc


def _get_program():
    global _PROGRAM
    if _PROGRAM is None:
        _PROGRAM = _build_program()
    return _PROGRAM


def _pos_emb(e0, e1):
    """float32 mirror of the reference _get_sample_point (one batch, sorted)."""
    angle = (e0 * F32(np.pi)).astype(F32)
    rho = (e1 * F32(IMG_W)).astype(F32)
    lin = np.linspace(0.0, 1.0 - 1e-5, NUM_OFFSETS, dtype=F32)
    yk = (F32(CENTER_H) - lin * F32(IMG_H)).astype(F32)[:2]
    tan = np.tan(angle, dtype=F32)
    roc = (rho / np.cos(angle, dtype=F32)).astype(F32)
    x = (-tan[:, None] * yk[None, :] + roc[:, None]).astype(F32)
    return (x / F32(IMG_W)).astype(F32)          # [n, 2]


def kernel(**inputs):
    bf = np.asarray(inputs["batch_features"], dtype=F32)      # [B,N,64]
    cls = np.asarray(inputs["cls_pred"], dtype=F32)           # [B,N]
    aid = np.asarray(inputs["anchor_id"])                     # [B,N] int32
    emb = np.asarray(inputs["anchor_embeddings"], dtype=F32)  # [B,N,2]

    w = {k: np.asarray(inputs[k], dtype=F32) for k in
         ("W_cls", "b_cls", "W_pos", "b_pos", "W_in", "b_in", "W_out", "b_out",
          "W_e1", "b_e1", "W_e2", "b_e2", "W_n1", "b_n1", "W_n2", "b_n2",
          "W_head", "b_head")}
    # A = feats@W_in + pos@W_pos + (b_in + b_pos); C = feats@W_out + b_out
    # + pos@W_pos.  Device omits b_out in C; fold it into be1:
    # qneg = b_e1 - q = (b_e1 - b_out@W_e1) - (C - b_out)@W_e1.
    bpos_eff = (w["b_in"] + w["b_pos"]).astype(F32)
    be1_eff = (w["b_e1"] - w["b_out"] @ w["W_e1"]).astype(F32)

    nc = _get_program()
    from concourse.bass_utils import run_bass_kernel_spmd

    iota = np.arange(N, dtype=F32)[None, :]
    we2d = np.zeros((I_DIM, TJ * TJ), dtype=F32)
    for j in range(TJ):
        we2d[:, TJ * j + j] = w["W_e2"][:, 0]
    # device nm_flat order: f = 8*pp + q  <->  local j index l = 32*q + pp
    l_of_f = np.array([TJ * q + pp for pp in range(TJ) for q in range(N_TILES)])

    shared = {
        "iota": iota, "we2d": we2d,
        "W_cls": w["W_cls"], "bcls": w["b_cls"][:, None],
        "W_in": w["W_in"], "W_out": w["W_out"], "W_pos": w["W_pos"],
        "bpos": bpos_eff[:, None], "W_e1": w["W_e1"],
        "be1": be1_eff[:, None], "we2": w["W_e2"],
        "be2c": np.full((128, 1), w["b_e2"][0], dtype=F32),
        "W_n1": w["W_n1"], "bn1": w["b_n1"][:, None],
        "W_n2": w["W_n2"], "bn2": w["b_n2"][:, None],
        "W_head": w["W_head"], "bh": w["b_head"][:, None],
    }

    in_maps = []
    perms = []
    rank_lists = []
    for b in range(B):
        perm = np.lexsort((-aid[b].astype(np.int64), -cls[b]))
        perms.append(perm)
        bf_s = bf[b][perm]                    # [N, 64]
        cls_s = cls[b][perm]
        e0_s = emb[b][perm, 0]
        e1_s = emb[b][perm, 1]
        ang_s = (e0_s * F32(np.pi)).astype(F32)
        pos_s = _pos_emb(e0_s, e1_s)          # [N, 2]

        bfT_i = np.ascontiguousarray(bf_s.T)
        posT_i = np.ascontiguousarray(pos_s.T)

        for P in range(2):
            ranks = np.concatenate(
                [np.arange(TJ * (2 * t + P), TJ * (2 * t + P) + TJ)
                 for t in range(N_TILES)])
            rank_lists.append(ranks[l_of_f])
            ang_loc = ang_s[ranks]
            m = dict(shared)
            m.update({
                "bfT_i": bfT_i,
                "posT_i": posT_i,
                "bfT_j": np.ascontiguousarray(bf_s[ranks].T),
                "posT_j": np.ascontiguousarray(pos_s[ranks].T),
                "angrow": ang_s[None, :],
                "angcol": np.ascontiguousarray(
                    np.stack([ang_loc[:128], ang_loc[128:]], axis=1)),
                "rankcol": np.ascontiguousarray(
                    np.stack([ranks[:128].astype(F32),
                              ranks[128:].astype(F32)], axis=1)),
                "cls_loc": cls_s[ranks[l_of_f]][None, :],
            })
            in_maps.append(m)

    res = run_bass_kernel_spmd(nc, in_maps, list(range(N_CORES)))

    out = np.zeros((B, N), dtype=F32)
    for ci in range(N_CORES):
        b = ci // 2
        probs = res.results[ci]["y"][0]       # [256] in core-local j order
        out[b, perms[b][rank_lists[ci]]] = probs
    return out



# revision 17
# speedup vs baseline: 2.9817x; 2.9817x over previous
"""Trainium2 Bass kernel for the O2O classification head (GNN message passing).

Strategy
--------
The reference edge tensor is rank-structured:
    edge[b,i,j,:] = A_i - C_j
with A = feats@W_in + pos@W_pos + (b_in+b_pos), C = feats@W_out + pos@W_pos
(+ b_out folded into be1).  After the first edge MLP layer the pre-gelu
values are p_i - q_j + b_e1 with p = A@W_e1, q = C@W_e1 computed once per
node, so each (i,j) pair costs one 128-wide gelu + dot with W_e2.

Host-side, nodes are sorted by (cls desc, id desc); suppress[i,j] != 0
requires rank_i < rank_j, so only i-prefixes matter.  Outputs for j with
cls_pred < 0.4 are exactly sigmoid(-1e6) == 0, so those j-columns are
pruned entirely: only the first K = #(cls >= 0.4) ranks are processed.

j-columns are tiled in blocks of TJ=16.  The G = ceil(K/16) blocks are
paired descending ((G-1,G-2), (G-3,G-4), ...) into T = ceil(G/2) slots;
the two cores of a batch take one block of each pair, and slot s uses an
i-prefix of L_s = 16*(G-2s).  All cores run an identical program; the
per-core variation is input data only.

The edge pipeline runs in bf16.  Per slot: one broadcast tensor_tensor
builds U (DVE), a single batched gelu (ACT), 16 diagonal-expanded
matmuls compute the W_e2 dot (PE), the additive suppression mask
(be2 where allowed, -1000 elsewhere; host-precomputed) is accumulated
into PSUM by one identity matmul, and the DVE does a max-reduce
(deferred one slot so it never stalls on the PE).  node_max is clamped
at 0 (the reference max always sees masked zeros).  The final sigmoid
is computed as 0.5*(1+tanh(x/2)): tanh shares the gelu activation-table
family, avoiding an ACT table reload.  Inputs are packed with no dead
rows and spread over four engine DMA queues to minimize head latency.
"""

import math
import sys

import numpy as np

if "/opt/trn_rl_repo" not in sys.path:
    sys.path.insert(0, "/opt/trn_rl_repo")

from ml_dtypes import bfloat16 as BF16  # noqa: E402

B, N = 4, 512
H_DIM, I_DIM = 64, 128
N_CORES = 8
TJ = 16                  # j's per slot
F32 = np.float32
NEG = -1000.0            # additive mask for suppressed entries

IMG_W, IMG_H, CENTER_H = 800.0, 320.0, 160.0
NUM_OFFSETS = 72
CONF_THRES = 0.4

_PROGRAMS = {}           # G -> compiled Bacc
_LAST = None             # (nc, in_maps) of the last kernel() call


def _plan(Kmax):
    G = max(1, -(-Kmax // TJ))           # number of 16-j rank blocks
    T = -(-G // 2)                       # slots per core
    L = [TJ * (G - 2 * s) for s in range(T)]   # i-prefix per slot
    NI = TJ * G                          # i-side length
    NJ = TJ * T                          # j's per core
    return G, T, L, NI, NJ


def _build_program(G, num_devices=N_CORES):
    import contextlib

    import concourse.bass as bass  # noqa: F401
    import concourse.tile as tile
    from concourse import bacc, mybir

    f32 = mybir.dt.float32
    bf16 = mybir.dt.bfloat16
    AF = mybir.ActivationFunctionType
    OP = mybir.AluOpType
    AX = mybir.AxisListType

    T = -(-G // 2)
    L = [TJ * (G - 2 * s) for s in range(T)]
    NI = TJ * G
    NJ = TJ * T
    SL = sum(L)
    OFF = [0] * T
    for s in range(1, T):
        OFF[s] = OFF[s - 1] + L[s - 1]

    nc = bacc.Bacc("TRN2", target_bir_lowering=False, debug=False,
                   num_devices=num_devices)

    bfpk = nc.declare_dram_parameter("bfpk", [64, NI + NJ], bf16,
                                     isOutput=False)
    w64pk = nc.declare_dram_parameter("w64pk", [64, 385], bf16,
                                      isOutput=False)
    w128pk = nc.declare_dram_parameter("w128pk", [128, 384], bf16,
                                       isOutput=False)
    pospk = nc.declare_dram_parameter("pospk", [2, NI + NJ + 192], bf16,
                                      isOutput=False)
    mpk = nc.declare_dram_parameter("mpk", [TJ, SL + TJ], f32,
                                    isOutput=False)
    bpk = nc.declare_dram_parameter("bpk", [128, 6], f32, isOutput=False)
    y = nc.declare_dram_parameter("y", [1, NJ], f32, isOutput=True)

    with tile.TileContext(nc) as tc:
        with contextlib.ExitStack() as ctx:
            const = ctx.enter_context(tc.tile_pool(name="const", bufs=1))
            work = ctx.enter_context(tc.tile_pool(name="work", bufs=2))
            upool = ctx.enter_context(tc.tile_pool(name="upool", bufs=3))
            gpool = ctx.enter_context(tc.tile_pool(name="gpool", bufs=3))
            pps = ctx.enter_context(tc.tile_pool(name="pps", bufs=2,
                                                 space="PSUM"))
            spsum = ctx.enter_context(tc.tile_pool(name="spsum", bufs=2,
                                                   space="PSUM"))

            # four DMA queues in parallel to minimize head latency
            sb_bf = const.tile([64, NI + NJ], bf16, name="sb_bf", tag="sb_bf")
            nc.gpsimd.dma_start(out=sb_bf[:], in_=bfpk[:])
            sb_w64 = const.tile([64, 385], bf16, name="sb_w64", tag="sb_w64")
            nc.sync.dma_start(out=sb_w64[:], in_=w64pk[:])
            sb_w128 = const.tile([128, 384], bf16, name="sb_w128",
                                 tag="sb_w128")
            nc.scalar.dma_start(out=sb_w128[:], in_=w128pk[:])
            sb_pos = const.tile([2, NI + NJ + 192], bf16, name="sb_pos",
                                tag="sb_pos")
            nc.scalar.dma_start(out=sb_pos[:], in_=pospk[:])
            sb_b = const.tile([128, 6], f32, name="sb_b", tag="sb_b")
            nc.sync.dma_start(out=sb_b[:], in_=bpk[:])
            sb_m = const.tile([TJ, SL + TJ], f32, name="sb_m", tag="sb_m")
            nc.sync.dma_start(out=sb_m[:], in_=mpk[:])

            bfT_i = sb_bf[0:64, 0:NI]
            bfT_j = sb_bf[0:64, NI:NI + NJ]
            W_cls = sb_w64[0:64, 0:64]
            W_in = sb_w64[0:64, 64:192]
            W_out = sb_w64[0:64, 192:320]
            W_n2 = sb_w64[0:64, 320:384]
            W_head = sb_w64[0:64, 384:385]
            W_e1 = sb_w128[0:128, 0:128]
            we2d = sb_w128[0:128, 128:384]
            posT_i = sb_pos[0:2, 0:NI]
            posT_j = sb_pos[0:2, NI:NI + NJ]
            W_pos = sb_pos[0:2, NI + NJ:NI + NJ + 128]
            W_n1 = sb_pos[0:1, NI + NJ + 128:NI + NJ + 192]
            ident16 = sb_m[0:TJ, SL:SL + TJ]
            bcls = sb_b[0:64, 0:1]
            bpos = sb_b[0:128, 1:2]
            be1 = sb_b[0:128, 2:3]
            bn1 = sb_b[0:64, 3:4]
            bn2 = sb_b[0:64, 4:5]
            bh2 = sb_b[0:1, 5:6]

            # ---------- i-side preprocessing (rank order, first NI) ---------
            ps1 = pps.tile([64, NI], f32, name="ps1", tag="ps")
            nc.tensor.matmul(ps1[:], W_cls, bfT_i, start=True, stop=True)
            featsT_i = const.tile([64, NI], bf16, name="featsT_i",
                                  tag="featsT_i")
            nc.vector.tensor_scalar(featsT_i[:], ps1[:], bcls, 0.0,
                                    OP.add, OP.max)

            ps2 = pps.tile([128, NI], f32, name="ps2", tag="ps")
            nc.tensor.matmul(ps2[:], W_in, featsT_i[:], start=True, stop=False)
            nc.tensor.matmul(ps2[:], W_pos, posT_i, start=False, stop=True)
            A_T = const.tile([128, NI], bf16, name="A_T", tag="A_T")
            nc.vector.tensor_scalar_add(A_T[:], ps2[:], bpos)

            ps3 = pps.tile([128, NI], f32, name="ps3", tag="ps")
            nc.tensor.matmul(ps3[:], W_e1, A_T[:], start=True, stop=True)
            p_T = const.tile([128, NI], bf16, name="p_T", tag="p_T")
            nc.vector.tensor_copy(p_T[:], ps3[:])

            # ---------- j-side preprocessing (core-local slot order) --------
            ps4 = pps.tile([64, NJ], f32, name="ps4", tag="ps")
            nc.tensor.matmul(ps4[:], W_cls, bfT_j, start=True, stop=True)
            featsT_j = const.tile([64, NJ], bf16, name="featsT_j",
                                  tag="featsT_j")
            nc.vector.tensor_scalar(featsT_j[:], ps4[:], bcls, 0.0,
                                    OP.add, OP.max)

            ps5 = pps.tile([128, NJ], f32, name="ps5", tag="ps")
            nc.tensor.matmul(ps5[:], W_out, featsT_j[:], start=True,
                             stop=False)
            nc.tensor.matmul(ps5[:], W_pos, posT_j, start=False, stop=True)
            C_T = const.tile([128, NJ], bf16, name="C_T", tag="C_T")
            nc.vector.tensor_copy(C_T[:], ps5[:])

            ps6 = pps.tile([128, NJ], f32, name="ps6", tag="ps")
            nc.tensor.matmul(ps6[:], W_e1, C_T[:], start=True, stop=True)
            qneg = const.tile([128, NJ], f32, name="qneg", tag="qneg")
            nc.vector.tensor_scalar(qneg[:], ps6[:], -1.0, be1,
                                    OP.mult, OP.add)

            # ---------- main loop (software-pipelined) ----------------------
            nmall = const.tile([TJ, T], bf16, name="nmall", tag="nmall")
            S_tiles = [None] * T

            def emit_reduce(s2_):
                nc.vector.reduce_max(nmall[:, s2_:s2_ + 1], S_tiles[s2_][:],
                                     axis=AX.X)

            for s in range(T):
                Ls = L[s]
                U = upool.tile([128, TJ * Ls], bf16, name=f"U_{s}", tag="u")
                # one broadcast add per slot: U[k, (j,i)] = p[k,i] + qneg[k,j]
                U3 = U[:].rearrange("p (j i) -> p j i", j=TJ)
                p3 = p_T[:, :Ls].unsqueeze(1).broadcast_to([128, TJ, Ls])
                q3 = qneg[:, s * TJ:(s + 1) * TJ].unsqueeze(2).broadcast_to(
                    [128, TJ, Ls])
                nc.vector.tensor_tensor(U3, p3, q3, OP.add)
                Gt = gpool.tile([128, TJ * Ls], bf16, name=f"G_{s}", tag="g")
                nc.scalar.activation(Gt[:], U[:], AF.Gelu)
                S = spsum.tile([TJ, Ls], f32, name=f"S_{s}", tag="sbank")
                for r in range(TJ):
                    nc.tensor.matmul(S[:, :], we2d[:, TJ * r:TJ * (r + 1)],
                                     Gt[:, r * Ls:(r + 1) * Ls],
                                     start=(r == 0), stop=False)
                # accumulate the additive mask on the PE so the DVE only
                # has to max-reduce: S += I16^T @ mask_slot
                nc.tensor.matmul(S[:, :], ident16,
                                 sb_m[0:TJ, OFF[s]:OFF[s] + Ls],
                                 start=False, stop=True)
                S_tiles[s] = S
                if s > 0:
                    emit_reduce(s - 1)
            emit_reduce(T - 1)

            nmc = const.tile([TJ, T], bf16, name="nmc", tag="nmc")
            nc.vector.tensor_scalar_max(nmc[:], nmall[:], 0.0)

            # ---------- final MLP over node_max -----------------------------
            # flatten [TJ, T] -> [1, NJ]; f = r*T + s (partition-major)
            nm_flat = work.tile([1, NJ], bf16, name="nm_flat", tag="nm_flat")
            nc.gpsimd.dma_start(out=nm_flat[:], in_=nmc[:])

            ph1 = pps.tile([64, NJ], f32, name="ph1", tag="ps")
            nc.tensor.matmul(ph1[:], W_n1, nm_flat[:], start=True, stop=True)
            s1 = work.tile([64, NJ], bf16, name="s1", tag="s1")
            nc.vector.tensor_scalar(s1[:], ph1[:], bn1, 0.0, OP.add, OP.max)

            ph2 = pps.tile([64, NJ], f32, name="ph2", tag="ps")
            nc.tensor.matmul(ph2[:], W_n2, s1[:], start=True, stop=True)
            s2 = work.tile([64, NJ], bf16, name="s2", tag="s2")
            nc.vector.tensor_scalar(s2[:], ph2[:], bn2, 0.0, OP.add, OP.max)

            phL = pps.tile([1, NJ], f32, name="phL", tag="ps")
            nc.tensor.matmul(phL[:], W_head, s2[:], start=True, stop=True)
            # sigmoid(x + bh) = 0.5*(1 + tanh((x + bh)/2)); tanh shares the
            # gelu activation table family (no ACT table reload).
            th = work.tile([1, NJ], f32, name="th", tag="th")
            nc.scalar.activation(th[:], phL[:], AF.Tanh, bias=bh2, scale=0.5)
            outp = work.tile([1, NJ], f32, name="outp", tag="outp")
            nc.vector.tensor_scalar(outp[:], th[:], 0.5, 0.5,
                                    OP.mult, OP.add)
            nc.gpsimd.dma_start(out=y[:], in_=outp[:])

    nc.compile()
    return nc


def _get_program(G=None):
    global _PROGRAMS
    if G is None:
        assert _PROGRAMS, "kernel() must run first"
        G = next(reversed(_PROGRAMS))
    if G not in _PROGRAMS:
        _PROGRAMS[G] = _build_program(G)
    return _PROGRAMS[G]


def _pos_emb(e0, e1):
    """float32 mirror of the reference _get_sample_point (one batch, sorted)."""
    angle = (e0 * F32(np.pi)).astype(F32)
    rho = (e1 * F32(IMG_W)).astype(F32)
    lin = np.linspace(0.0, 1.0 - 1e-5, NUM_OFFSETS, dtype=F32)
    yk = (F32(CENTER_H) - lin * F32(IMG_H)).astype(F32)[:2]
    tan = np.tan(angle, dtype=F32)
    roc = (rho / np.cos(angle, dtype=F32)).astype(F32)
    x = (-tan[:, None] * yk[None, :] + roc[:, None]).astype(F32)
    return (x / F32(IMG_W)).astype(F32)          # [n, 2]


def _host_prepare(inputs):
    """Sort, prune, tile; returns (G, in_maps, scatter) where scatter is a
    list of (b, targets, f_idx) per core mapping device outputs to [B, N]."""
    bf = np.asarray(inputs["batch_features"], dtype=F32)      # [B,N,64]
    cls = np.asarray(inputs["cls_pred"], dtype=F32)           # [B,N]
    aid = np.asarray(inputs["anchor_id"])                     # [B,N] int32
    emb = np.asarray(inputs["anchor_embeddings"], dtype=F32)  # [B,N,2]

    w = {k: np.asarray(inputs[k], dtype=F32) for k in
         ("W_cls", "b_cls", "W_pos", "b_pos", "W_in", "b_in", "W_out",
          "b_out", "W_e1", "b_e1", "W_e2", "b_e2", "W_n1", "b_n1", "W_n2",
          "b_n2", "W_head", "b_head")}

    Kb = (cls >= F32(CONF_THRES)).sum(axis=1)
    Kmax = int(Kb.max())
    if Kmax == 0:
        return None
    G, T, L, NI, NJ = _plan(Kmax)
    SL = sum(L)
    OFF = np.concatenate([[0], np.cumsum(L)[:-1]]).astype(np.int64)

    # device folds b_out into be1: qneg = (b_e1 - b_out@W_e1) - C'@W_e1
    bpos_eff = (w["b_in"] + w["b_pos"]).astype(F32)
    be1_eff = (w["b_e1"] - w["b_out"] @ w["W_e1"]).astype(F32)
    be2 = F32(w["b_e2"][0])

    we2d = np.zeros((I_DIM, TJ * TJ), dtype=F32)
    for j in range(TJ):
        we2d[:, TJ * j + j] = w["W_e2"][:, 0]

    w64pk = np.zeros((64, 385), dtype=F32)
    w64pk[:, 0:64] = w["W_cls"]
    w64pk[:, 64:192] = w["W_in"]
    w64pk[:, 192:320] = w["W_out"]
    w64pk[:, 320:384] = w["W_n2"]
    w64pk[:, 384:385] = w["W_head"]
    w64pk = w64pk.astype(BF16)

    w128pk = np.concatenate([w["W_e1"], we2d], axis=1).astype(BF16)

    bpk = np.zeros((128, 6), dtype=F32)
    bpk[0:64, 0] = w["b_cls"]
    bpk[0:128, 1] = bpos_eff
    bpk[0:128, 2] = be1_eff
    bpk[0:64, 3] = w["b_n1"]
    bpk[0:64, 4] = w["b_n2"]
    bpk[0:1, 5] = w["b_head"] / 2.0

    in_maps, scatter = [], []
    for b in range(B):
        perm = np.lexsort((-aid[b].astype(np.int64), -cls[b]))
        bf_s = bf[b][perm]                     # [N, 64]
        e0_s = emb[b][perm, 0]
        e1_s = emb[b][perm, 1]
        ang_s = (e0_s * F32(np.pi)).astype(F32)
        pos_s = _pos_emb(e0_s, e1_s)           # [N, 2]

        for c in range(2):
            blocks = [G - 1 - 2 * s - c for s in range(T)]
            jr = np.empty((T, TJ), dtype=np.int64)
            valid = np.empty(T, dtype=bool)
            for s, g in enumerate(blocks):
                gg = g if g >= 0 else 0
                jr[s] = TJ * gg + np.arange(TJ)
                valid[s] = g >= 0
            jr_flat = jr.reshape(-1)           # l = s*TJ + r

            bfpk = np.concatenate(
                [bf_s[:NI].T, bf_s[jr_flat].T], axis=1).astype(BF16)
            pospk = np.zeros((2, NI + NJ + 192), dtype=F32)
            pospk[:, 0:NI] = pos_s[:NI].T
            pospk[:, NI:NI + NJ] = pos_s[jr_flat].T
            pospk[:, NI + NJ:NI + NJ + 128] = w["W_pos"]
            pospk[0, NI + NJ + 128:NI + NJ + 192] = w["W_n1"][0]
            pospk = pospk.astype(BF16)

            # additive suppression masks, host-built:
            # M = be2 where (|ang_i - ang_j| < 0.5 and rank_i < rank_j)
            mpk = np.full((TJ, SL + TJ), F32(NEG), dtype=F32)
            for s in range(T):
                Ls = L[s]
                aj = ang_s[jr[s]]                       # [TJ]
                ai = ang_s[:Ls]                         # [Ls]
                ok = np.abs(aj[:, None] - ai[None, :]) < F32(0.5)
                if valid[s]:
                    ok &= np.arange(Ls)[None, :] < jr[s][:, None]
                else:
                    ok &= False
                mpk[:, OFF[s]:OFF[s] + Ls] = np.where(ok, be2, F32(NEG))
            mpk[:, SL:] = np.eye(TJ, dtype=F32)

            in_maps.append({"bfpk": bfpk, "w64pk": w64pk, "w128pk": w128pk,
                            "pospk": pospk, "mpk": mpk, "bpk": bpk})

            # output scatter: f = r*T + s -> rank jr[s, r] (if valid+conf)
            f_idx, tgt = [], []
            for s in range(T):
                if not valid[s]:
                    continue
                for r in range(TJ):
                    rank = jr[s, r]
                    if rank < Kb[b]:
                        f_idx.append(r * T + s)
                        tgt.append(perm[rank])
            scatter.append((b, np.asarray(tgt, dtype=np.int64),
                            np.asarray(f_idx, dtype=np.int64)))

    return G, in_maps, scatter


def kernel(**inputs):
    global _LAST
    out = np.zeros((B, N), dtype=F32)
    prep = _host_prepare(inputs)
    if prep is None:
        return out
    G, in_maps, scatter = prep

    nc = _get_program(G)
    from concourse.bass_utils import run_bass_kernel_spmd

    res = run_bass_kernel_spmd(nc, in_maps, list(range(N_CORES)))
    _LAST = (nc, in_maps)

    for ci in range(N_CORES):
        b, tgt, f_idx = scatter[ci]
        probs = res.results[ci]["y"][0]        # [NJ]
        out[b, tgt] = probs[f_idx]
    return out


# revision 19
# speedup vs baseline: 3.0268x; 1.0151x over previous
"""Trainium2 Bass kernel for the O2O classification head (GNN message passing).

Strategy
--------
The reference edge tensor is rank-structured:
    edge[b,i,j,:] = A_i - C_j
with A = feats@W_in + pos@W_pos + (b_in+b_pos), C = feats@W_out + pos@W_pos
(+ b_out folded into be1).  After the first edge MLP layer the pre-gelu
values are p_i - q_j + b_e1 with p = A@W_e1, q = C@W_e1 computed once per
node, so each (i,j) pair costs one 128-wide gelu + dot with W_e2.

Host-side, nodes are sorted by (cls desc, id desc); suppress[i,j] != 0
requires rank_i < rank_j, so only i-prefixes matter.  Outputs for j with
cls_pred < 0.4 are exactly sigmoid(-1e6) == 0, so those j-columns are
pruned entirely: only the first K = #(cls >= 0.4) ranks are processed.

j-columns are tiled in blocks of TJ=16.  The G = ceil(K/16) blocks are
paired descending ((G-1,G-2), (G-3,G-4), ...) into T = ceil(G/2) slots;
the two cores of a batch take one block of each pair, and slot s uses an
i-prefix of L_s = 16*(G-2s).  All cores run an identical program; the
per-core variation is input data only.

The edge pipeline runs in bf16.  Per slot: one broadcast tensor_tensor
builds U (DVE), a single batched gelu (ACT), 16 diagonal-expanded
matmuls compute the W_e2 dot (PE), the additive suppression mask
(be2 where allowed, -1000 elsewhere; host-precomputed) is accumulated
into PSUM by one identity matmul, and the DVE does a max-reduce
(deferred one slot so it never stalls on the PE).  node_max is clamped
at 0 (the reference max always sees masked zeros).  The final sigmoid
is computed as 0.5*(1+tanh(x/2)): tanh shares the gelu activation-table
family, avoiding an ACT table reload.  Inputs are packed with no dead
rows and spread over four engine DMA queues to minimize head latency.
"""

import math
import sys

import numpy as np

if "/opt/trn_rl_repo" not in sys.path:
    sys.path.insert(0, "/opt/trn_rl_repo")

from ml_dtypes import bfloat16 as BF16  # noqa: E402

B, N = 4, 512
H_DIM, I_DIM = 64, 128
N_CORES = 8
TJ = 16                  # j's per slot
F32 = np.float32
NEG = -1000.0            # additive mask for suppressed entries

IMG_W, IMG_H, CENTER_H = 800.0, 320.0, 160.0
NUM_OFFSETS = 72
CONF_THRES = 0.4

_PROGRAMS = {}           # G -> compiled Bacc
_LAST = None             # (nc, in_maps) of the last kernel() call


def _plan(Kmax):
    G = max(1, -(-Kmax // TJ))           # number of 16-j rank blocks
    T = -(-G // 2)                       # slots per core
    L = [TJ * (G - 2 * s) for s in range(T)]   # i-prefix per slot
    NI = TJ * G                          # i-side length
    NJ = TJ * T                          # j's per core
    return G, T, L, NI, NJ


def _build_program(G, num_devices=N_CORES):
    import contextlib

    import concourse.bass as bass  # noqa: F401
    import concourse.tile as tile
    from concourse import bacc, mybir

    f32 = mybir.dt.float32
    bf16 = mybir.dt.bfloat16
    AF = mybir.ActivationFunctionType
    OP = mybir.AluOpType
    AX = mybir.AxisListType

    T = -(-G // 2)
    L = [TJ * (G - 2 * s) for s in range(T)]
    NI = TJ * G
    NJ = TJ * T
    SL = sum(L)
    OFF = [0] * T
    for s in range(1, T):
        OFF[s] = OFF[s - 1] + L[s - 1]

    nc = bacc.Bacc("TRN2", target_bir_lowering=False, debug=False,
                   num_devices=num_devices)

    bfpk = nc.declare_dram_parameter("bfpk", [64, NI + NJ], bf16,
                                     isOutput=False)
    w64pk = nc.declare_dram_parameter("w64pk", [64, 385], bf16,
                                      isOutput=False)
    w128pk = nc.declare_dram_parameter("w128pk", [128, 384], bf16,
                                       isOutput=False)
    pospk = nc.declare_dram_parameter("pospk", [2, NI + NJ + 192], bf16,
                                      isOutput=False)
    mpk = nc.declare_dram_parameter("mpk", [TJ, SL + TJ], f32,
                                    isOutput=False)
    bpk = nc.declare_dram_parameter("bpk", [128, 6], f32, isOutput=False)
    y = nc.declare_dram_parameter("y", [1, NJ], f32, isOutput=True)

    with tile.TileContext(nc) as tc:
        with contextlib.ExitStack() as ctx:
            const = ctx.enter_context(tc.tile_pool(name="const", bufs=1))
            work = ctx.enter_context(tc.tile_pool(name="work", bufs=2))
            upool = ctx.enter_context(tc.tile_pool(name="upool", bufs=3))
            gpool = ctx.enter_context(tc.tile_pool(name="gpool", bufs=3))
            pps = ctx.enter_context(tc.tile_pool(name="pps", bufs=2,
                                                 space="PSUM"))
            spsum = ctx.enter_context(tc.tile_pool(name="spsum", bufs=2,
                                                   space="PSUM"))

            # four DMA queues in parallel to minimize head latency
            sb_bf = const.tile([64, NI + NJ], bf16, name="sb_bf", tag="sb_bf")
            nc.scalar.dma_start(out=sb_bf[:], in_=bfpk[:])
            sb_w64 = const.tile([64, 385], bf16, name="sb_w64", tag="sb_w64")
            nc.sync.dma_start(out=sb_w64[:], in_=w64pk[:])
            sb_w128 = const.tile([128, 384], bf16, name="sb_w128",
                                 tag="sb_w128")
            nc.scalar.dma_start(out=sb_w128[:], in_=w128pk[:])
            sb_pos = const.tile([2, NI + NJ + 192], bf16, name="sb_pos",
                                tag="sb_pos")
            nc.scalar.dma_start(out=sb_pos[:], in_=pospk[:])
            sb_b = const.tile([128, 6], f32, name="sb_b", tag="sb_b")
            nc.sync.dma_start(out=sb_b[:], in_=bpk[:])
            sb_m = const.tile([TJ, SL + TJ], f32, name="sb_m", tag="sb_m")
            nc.sync.dma_start(out=sb_m[:], in_=mpk[:])

            bfT_i = sb_bf[0:64, 0:NI]
            bfT_j = sb_bf[0:64, NI:NI + NJ]
            W_cls = sb_w64[0:64, 0:64]
            W_in = sb_w64[0:64, 64:192]
            W_out = sb_w64[0:64, 192:320]
            W_n2 = sb_w64[0:64, 320:384]
            W_head = sb_w64[0:64, 384:385]
            W_e1 = sb_w128[0:128, 0:128]
            we2d = sb_w128[0:128, 128:384]
            posT_i = sb_pos[0:2, 0:NI]
            posT_j = sb_pos[0:2, NI:NI + NJ]
            W_pos = sb_pos[0:2, NI + NJ:NI + NJ + 128]
            W_n1 = sb_pos[0:1, NI + NJ + 128:NI + NJ + 192]
            ident16 = sb_m[0:TJ, SL:SL + TJ]
            bcls = sb_b[0:64, 0:1]
            bpos = sb_b[0:128, 1:2]
            be1 = sb_b[0:128, 2:3]
            bn1 = sb_b[0:64, 3:4]
            bn2 = sb_b[0:64, 4:5]
            bh2 = sb_b[0:1, 5:6]

            # ---------- preprocessing (elementwise on ACT; DVE untouched) ---
            ps1 = pps.tile([64, NI], f32, name="ps1", tag="ps")
            nc.tensor.matmul(ps1[:], W_cls, bfT_i, start=True, stop=True)
            featsT_i = const.tile([64, NI], bf16, name="featsT_i",
                                  tag="featsT_i")
            nc.scalar.activation(featsT_i[:], ps1[:], AF.Relu, bias=bcls)

            ps2 = pps.tile([128, NI], f32, name="ps2", tag="ps")
            nc.tensor.matmul(ps2[:], W_in, featsT_i[:], start=True, stop=False)
            nc.tensor.matmul(ps2[:], W_pos, posT_i, start=False, stop=True)
            A_T = const.tile([128, NI], bf16, name="A_T", tag="A_T")
            nc.scalar.activation(A_T[:], ps2[:], AF.Identity, bias=bpos)

            # p stays in PSUM (its own bank); the broadcast-add reads it
            ppq = ctx.enter_context(tc.tile_pool(name="ppq", bufs=1,
                                                 space="PSUM"))
            ps3 = ppq.tile([128, NI], f32, name="ps3", tag="ps3")
            nc.tensor.matmul(ps3[:], W_e1, A_T[:], start=True, stop=True)

            ps4 = pps.tile([64, NJ], f32, name="ps4", tag="ps")
            nc.tensor.matmul(ps4[:], W_cls, bfT_j, start=True, stop=True)
            featsT_j = const.tile([64, NJ], bf16, name="featsT_j",
                                  tag="featsT_j")
            nc.scalar.activation(featsT_j[:], ps4[:], AF.Relu, bias=bcls)

            ps5 = pps.tile([128, NJ], f32, name="ps5", tag="ps")
            nc.tensor.matmul(ps5[:], W_out, featsT_j[:], start=True,
                             stop=False)
            nc.tensor.matmul(ps5[:], W_pos, posT_j, start=False, stop=True)
            C_T = const.tile([128, NJ], bf16, name="C_T", tag="C_T")
            nc.scalar.activation(C_T[:], ps5[:], AF.Identity)

            ps6 = pps.tile([128, NJ], f32, name="ps6", tag="ps")
            nc.tensor.matmul(ps6[:], W_e1, C_T[:], start=True, stop=True)
            qneg = const.tile([128, NJ], f32, name="qneg", tag="qneg")
            # qneg = be1 - q  ==  Identity(ps6 * -1 + be1)
            nc.scalar.activation(qneg[:], ps6[:], AF.Identity, bias=be1,
                                 scale=-1.0)

            # ---------- main loop (software-pipelined) ----------------------
            nmall = const.tile([TJ, T], bf16, name="nmall", tag="nmall")
            S_tiles = [None] * T

            def emit_reduce(s2_):
                nc.vector.reduce_max(nmall[:, s2_:s2_ + 1], S_tiles[s2_][:],
                                     axis=AX.X)

            for s in range(T):
                Ls = L[s]
                U = upool.tile([128, TJ * Ls], bf16, name=f"U_{s}", tag="u")
                # one broadcast add per slot: U[k, (j,i)] = p[k,i] + qneg[k,j]
                U3 = U[:].rearrange("p (j i) -> p j i", j=TJ)
                p3 = ps3[:, :Ls].unsqueeze(1).broadcast_to([128, TJ, Ls])
                q3 = qneg[:, s * TJ:(s + 1) * TJ].unsqueeze(2).broadcast_to(
                    [128, TJ, Ls])
                nc.vector.tensor_tensor(U3, p3, q3, OP.add)
                Gt = gpool.tile([128, TJ * Ls], bf16, name=f"G_{s}", tag="g")
                nc.scalar.activation(Gt[:], U[:], AF.Gelu)
                S = spsum.tile([TJ, Ls], f32, name=f"S_{s}", tag="sbank")
                for r in range(TJ):
                    nc.tensor.matmul(S[:, :], we2d[:, TJ * r:TJ * (r + 1)],
                                     Gt[:, r * Ls:(r + 1) * Ls],
                                     start=(r == 0), stop=False)
                # accumulate the additive mask on the PE so the DVE only
                # has to max-reduce: S += I16^T @ mask_slot
                nc.tensor.matmul(S[:, :], ident16,
                                 sb_m[0:TJ, OFF[s]:OFF[s] + Ls],
                                 start=False, stop=True)
                S_tiles[s] = S
                if s > 0:
                    emit_reduce(s - 1)
            emit_reduce(T - 1)

            nmc = const.tile([TJ, T], bf16, name="nmc", tag="nmc")
            nc.scalar.activation(nmc[:], nmall[:], AF.Relu)

            # ---------- final MLP over node_max -----------------------------
            # flatten [TJ, T] -> [1, NJ]; f = r*T + s (partition-major)
            nm_flat = work.tile([1, NJ], bf16, name="nm_flat", tag="nm_flat")
            nc.sync.dma_start(out=nm_flat[:], in_=nmc[:])

            ph1 = pps.tile([64, NJ], f32, name="ph1", tag="ps")
            nc.tensor.matmul(ph1[:], W_n1, nm_flat[:], start=True, stop=True)
            s1 = work.tile([64, NJ], bf16, name="s1", tag="s1")
            nc.scalar.activation(s1[:], ph1[:], AF.Relu, bias=bn1)

            ph2 = pps.tile([64, NJ], f32, name="ph2", tag="ps")
            nc.tensor.matmul(ph2[:], W_n2, s1[:], start=True, stop=True)
            s2 = work.tile([64, NJ], bf16, name="s2", tag="s2")
            nc.scalar.activation(s2[:], ph2[:], AF.Relu, bias=bn2)

            phL = pps.tile([1, NJ], f32, name="phL", tag="ps")
            nc.tensor.matmul(phL[:], W_head, s2[:], start=True, stop=True)
            # sigmoid(x + bh) = 0.5*(1 + tanh((x + bh)/2)); tanh shares the
            # gelu activation table family (no ACT table reload).
            th = work.tile([1, NJ], f32, name="th", tag="th")
            nc.scalar.activation(th[:], phL[:], AF.Tanh, bias=bh2, scale=0.5)
            outp = work.tile([1, NJ], f32, name="outp", tag="outp")
            nc.scalar.activation(outp[:], th[:], AF.Copy, bias=0.5, scale=0.5)
            nc.sync.dma_start(out=y[:], in_=outp[:])

    nc.compile()
    return nc


def _get_program(G=None):
    global _PROGRAMS
    if G is None:
        assert _PROGRAMS, "kernel() must run first"
        G = next(reversed(_PROGRAMS))
    if G not in _PROGRAMS:
        _PROGRAMS[G] = _build_program(G)
    return _PROGRAMS[G]


def _pos_emb(e0, e1):
    """float32 mirror of the reference _get_sample_point (one batch, sorted)."""
    angle = (e0 * F32(np.pi)).astype(F32)
    rho = (e1 * F32(IMG_W)).astype(F32)
    lin = np.linspace(0.0, 1.0 - 1e-5, NUM_OFFSETS, dtype=F32)
    yk = (F32(CENTER_H) - lin * F32(IMG_H)).astype(F32)[:2]
    tan = np.tan(angle, dtype=F32)
    roc = (rho / np.cos(angle, dtype=F32)).astype(F32)
    x = (-tan[:, None] * yk[None, :] + roc[:, None]).astype(F32)
    return (x / F32(IMG_W)).astype(F32)          # [n, 2]


def _host_prepare(inputs):
    """Sort, prune, tile; returns (G, in_maps, scatter) where scatter is a
    list of (b, targets, f_idx) per core mapping device outputs to [B, N]."""
    bf = np.asarray(inputs["batch_features"], dtype=F32)      # [B,N,64]
    cls = np.asarray(inputs["cls_pred"], dtype=F32)           # [B,N]
    aid = np.asarray(inputs["anchor_id"])                     # [B,N] int32
    emb = np.asarray(inputs["anchor_embeddings"], dtype=F32)  # [B,N,2]

    w = {k: np.asarray(inputs[k], dtype=F32) for k in
         ("W_cls", "b_cls", "W_pos", "b_pos", "W_in", "b_in", "W_out",
          "b_out", "W_e1", "b_e1", "W_e2", "b_e2", "W_n1", "b_n1", "W_n2",
          "b_n2", "W_head", "b_head")}

    Kb = (cls >= F32(CONF_THRES)).sum(axis=1)
    Kmax = int(Kb.max())
    if Kmax == 0:
        return None
    G, T, L, NI, NJ = _plan(Kmax)
    SL = sum(L)
    OFF = np.concatenate([[0], np.cumsum(L)[:-1]]).astype(np.int64)

    # device folds b_out into be1: qneg = (b_e1 - b_out@W_e1) - C'@W_e1
    bpos_eff = (w["b_in"] + w["b_pos"]).astype(F32)
    be1_eff = (w["b_e1"] - w["b_out"] @ w["W_e1"]).astype(F32)
    be2 = F32(w["b_e2"][0])

    we2d = np.zeros((I_DIM, TJ * TJ), dtype=F32)
    for j in range(TJ):
        we2d[:, TJ * j + j] = w["W_e2"][:, 0]

    w64pk = np.zeros((64, 385), dtype=F32)
    w64pk[:, 0:64] = w["W_cls"]
    w64pk[:, 64:192] = w["W_in"]
    w64pk[:, 192:320] = w["W_out"]
    w64pk[:, 320:384] = w["W_n2"]
    w64pk[:, 384:385] = w["W_head"]
    w64pk = w64pk.astype(BF16)

    w128pk = np.concatenate([w["W_e1"], we2d], axis=1).astype(BF16)

    bpk = np.zeros((128, 6), dtype=F32)
    bpk[0:64, 0] = w["b_cls"]
    bpk[0:128, 1] = bpos_eff
    bpk[0:128, 2] = be1_eff
    bpk[0:64, 3] = w["b_n1"]
    bpk[0:64, 4] = w["b_n2"]
    bpk[0:1, 5] = w["b_head"] / 2.0

    in_maps, scatter = [], []
    for b in range(B):
        perm = np.lexsort((-aid[b].astype(np.int64), -cls[b]))
        bf_s = bf[b][perm]                     # [N, 64]
        e0_s = emb[b][perm, 0]
        e1_s = emb[b][perm, 1]
        ang_s = (e0_s * F32(np.pi)).astype(F32)
        pos_s = _pos_emb(e0_s, e1_s)           # [N, 2]

        for c in range(2):
            blocks = [G - 1 - 2 * s - c for s in range(T)]
            jr = np.empty((T, TJ), dtype=np.int64)
            valid = np.empty(T, dtype=bool)
            for s, g in enumerate(blocks):
                gg = g if g >= 0 else 0
                jr[s] = TJ * gg + np.arange(TJ)
                valid[s] = g >= 0
            jr_flat = jr.reshape(-1)           # l = s*TJ + r

            bfpk = np.concatenate(
                [bf_s[:NI].T, bf_s[jr_flat].T], axis=1).astype(BF16)
            pospk = np.zeros((2, NI + NJ + 192), dtype=F32)
            pospk[:, 0:NI] = pos_s[:NI].T
            pospk[:, NI:NI + NJ] = pos_s[jr_flat].T
            pospk[:, NI + NJ:NI + NJ + 128] = w["W_pos"]
            pospk[0, NI + NJ + 128:NI + NJ + 192] = w["W_n1"][0]
            pospk = pospk.astype(BF16)

            # additive suppression masks, host-built:
            # M = be2 where (|ang_i - ang_j| < 0.5 and rank_i < rank_j)
            mpk = np.full((TJ, SL + TJ), F32(NEG), dtype=F32)
            for s in range(T):
                Ls = L[s]
                aj = ang_s[jr[s]]                       # [TJ]
                ai = ang_s[:Ls]                         # [Ls]
                ok = np.abs(aj[:, None] - ai[None, :]) < F32(0.5)
                if valid[s]:
                    ok &= np.arange(Ls)[None, :] < jr[s][:, None]
                else:
                    ok &= False
                mpk[:, OFF[s]:OFF[s] + Ls] = np.where(ok, be2, F32(NEG))
            mpk[:, SL:] = np.eye(TJ, dtype=F32)

            in_maps.append({"bfpk": bfpk, "w64pk": w64pk, "w128pk": w128pk,
                            "pospk": pospk, "mpk": mpk, "bpk": bpk})

            # output scatter: f = r*T + s -> rank jr[s, r] (if valid+conf)
            f_idx, tgt = [], []
            for s in range(T):
                if not valid[s]:
                    continue
                for r in range(TJ):
                    rank = jr[s, r]
                    if rank < Kb[b]:
                        f_idx.append(r * T + s)
                        tgt.append(perm[rank])
            scatter.append((b, np.asarray(tgt, dtype=np.int64),
                            np.asarray(f_idx, dtype=np.int64)))

    return G, in_maps, scatter


def kernel(**inputs):
    global _LAST
    out = np.zeros((B, N), dtype=F32)
    prep = _host_prepare(inputs)
    if prep is None:
        return out
    G, in_maps, scatter = prep

    nc = _get_program(G)
    from concourse.bass_utils import run_bass_kernel_spmd

    res = run_bass_kernel_spmd(nc, in_maps, list(range(N_CORES)))
    _LAST = (nc, in_maps)

    for ci in range(N_CORES):
        b, tgt, f_idx = scatter[ci]
        probs = res.results[ci]["y"][0]        # [NJ]
        out[b, tgt] = probs[f_idx]
    return out
